# revision 1
# baseline (speedup 1.0000x reference)
"""Trainium2 Bass kernel for CombinedVectorField (CFG vector field + exact
Jacobian-trace divergence).

Math: with u = tanh(x@W1x + h@W1h + b1'), b1' = b1 + t*W1[256],
  v(x,h)  = u @ W2 + b2
  div(x,h)= sum_k (1-u_k^2) c_k = d0 - (u*u) @ c,   c_k = sum_i W1x[i,k] W2[k,i]
Output = concat[(1-gs)*v_null + gs*v_h, (1-gs)*div_null + gs*div_h].

Sharding: pure data parallel — each of the 8 cores takes 512 batch rows
(both guidance branches), weights replicated. All tensors are kept
feature-major (transposed) on device so every matmul contracts over the
partition dim; host does the transposes/reshapes only.
"""
import sys

sys.path.insert(0, "/opt/trn_rl_repo")

import numpy as np

import concourse.bass as bass
import concourse.tile as tile
from concourse import bacc, mybir
from concourse.bass_utils import run_bass_kernel_spmd

F32 = mybir.dt.float32
F32R = mybir.dt.float32r
AF = mybir.ActivationFunctionType
ALU = mybir.AluOpType

N_CORES = 8
B = 4096
DIM_X = 128
DIM_H = 128
HIDDEN = 512
R = B // N_CORES          # rows per core
NCH = HIDDEN // 128       # hidden chunks

_NC_CACHE = None


def _build():
    nc = bacc.Bacc("TRN2", target_bir_lowering=False, debug=False)

    xT = nc.dram_tensor("xT", [DIM_X, R], F32R, kind="ExternalInput")
    hT = nc.dram_tensor("hT", [DIM_H, R], F32R, kind="ExternalInput")
    hnT = nc.dram_tensor("hnT", [DIM_H, R], F32R, kind="ExternalInput")
    w1x = nc.dram_tensor("w1x", [DIM_X, HIDDEN], F32R, kind="ExternalInput")
    w1h = nc.dram_tensor("w1h", [DIM_H, HIDDEN], F32R, kind="ExternalInput")
    # W2 pre-rearranged on host: w2r[k, c*128+i] = W2[c*128+k, i]
    w2t = nc.dram_tensor("w2t", [128, NCH * DIM_X], F32R, kind="ExternalInput")
    cmat = nc.dram_tensor("cmat", [128, NCH], F32R, kind="ExternalInput")
    # aux cols: 0-3 b1' chunks, 4 b2, 5 gs, 6 1-gs, 7 -(1-gs), 8 d0, 9 -gs
    aux = nc.dram_tensor("aux", [128, 10], F32, kind="ExternalInput")

    VO = nc.dram_tensor("VO", [DIM_X, R], F32, kind="ExternalOutput")
    DO = nc.dram_tensor("DO", [1, R], F32, kind="ExternalOutput")

    with tile.TileContext(nc) as tc:
        with tc.tile_pool(name="cst", bufs=1) as cst, \
             tc.tile_pool(name="act", bufs=3) as actp, \
             tc.tile_pool(name="out", bufs=1) as outp, \
             tc.tile_pool(name="psa", bufs=2, space="PSUM") as psa, \
             tc.tile_pool(name="psv", bufs=1, space="PSUM") as psv:
            auxt = cst.tile([128, 10], F32)
            nc.sync.dma_start(out=auxt[:], in_=aux[:])
            cmt = cst.tile([128, NCH], F32R)
            nc.sync.dma_start(out=cmt[:], in_=cmat[:])
            xt = cst.tile([DIM_X, R], F32R)
            nc.sync.dma_start(out=xt[:], in_=xT[:])
            ht = cst.tile([DIM_H, R], F32R)
            nc.sync.dma_start(out=ht[:], in_=hT[:])
            hnt = cst.tile([DIM_H, R], F32R)
            nc.sync.dma_start(out=hnt[:], in_=hnT[:])
            w1xt = cst.tile([DIM_X, HIDDEN], F32R)
            w1ht = cst.tile([DIM_H, HIDDEN], F32R)
            w2tt = cst.tile([128, NCH * DIM_X], F32R)
            for c in range(NCH):
                cs = bass.ts(c, 128)
                nc.sync.dma_start(out=w1xt[:, cs], in_=w1x[:, cs])
                nc.sync.dma_start(out=w1ht[:, cs], in_=w1h[:, cs])
                nc.sync.dma_start(out=w2tt[:, cs], in_=w2t[:, cs])

            pv = psv.tile([128, 2 * R], F32)       # v accum: [h | null]
            pdh = psv.tile([1, R], F32)            # sum c*u^2, branch h
            pdn = psv.tile([1, R], F32)            # sum c*u^2, branch null

            for c in range(NCH):
                cs = bass.ts(c, 128)
                first, last = c == 0, c == NCH - 1
                a = psa.tile([128, 2 * R], F32)    # pre-act: [h | null]
                nc.tensor.matmul(a[:, 0:R], w1xt[:, cs], xt[:], start=True, stop=False)
                nc.tensor.matmul(a[:, R:2 * R], w1xt[:, cs], xt[:], start=True, stop=False)
                nc.tensor.matmul(a[:, 0:R], w1ht[:, cs], ht[:], start=False, stop=True)
                nc.tensor.matmul(a[:, R:2 * R], w1ht[:, cs], hnt[:], start=False, stop=True)

                u = actp.tile([128, 2 * R], F32R, tag="u")
                nc.scalar.activation(u[:], a[:], AF.Tanh, bias=auxt[:, c:c + 1], scale=1.0)
                u2 = actp.tile([128, 2 * R], F32R, tag="u2")
                nc.vector.tensor_tensor(u2[:], u[:], u[:], op=ALU.mult)

                nc.tensor.matmul(pv[:, 0:R], w2tt[:, cs], u[:, 0:R], start=first, stop=last)
                nc.tensor.matmul(pv[:, R:2 * R], w2tt[:, cs], u[:, R:2 * R], start=first, stop=last)
                nc.tensor.matmul(pdh[0:1, :], cmt[:, c:c + 1], u2[:, 0:R], start=first, stop=last)
                nc.tensor.matmul(pdn[0:1, :], cmt[:, c:c + 1], u2[:, R:2 * R], start=first, stop=last)

            # v = gs*v_h + ((1-gs)*v_null + b2)
            t2 = outp.tile([128, R], F32)
            nc.vector.tensor_scalar(t2[:], pv[:, R:2 * R], auxt[:, 6:7], auxt[:, 4:5],
                                    op0=ALU.mult, op1=ALU.add)
            vout = outp.tile([128, R], F32)
            nc.vector.scalar_tensor_tensor(vout[:], pv[:, 0:R], auxt[:, 5:6], t2[:],
                                           op0=ALU.mult, op1=ALU.add)
            # div = d0 - gs*s_h - (1-gs)*s_n
            dt2 = outp.tile([1, R], F32)
            nc.vector.tensor_scalar(dt2[:], pdn[0:1, :], auxt[0:1, 7:8], auxt[0:1, 8:9],
                                    op0=ALU.mult, op1=ALU.add)
            dout = outp.tile([1, R], F32)
            nc.vector.scalar_tensor_tensor(dout[:], pdh[0:1, :], auxt[0:1, 9:10], dt2[:],
                                           op0=ALU.mult, op1=ALU.add)

            nc.sync.dma_start(out=VO[:], in_=vout[:])
            nc.sync.dma_start(out=DO[:], in_=dout[:])
    nc.compile()
    return nc


def _get_nc():
    global _NC_CACHE
    if _NC_CACHE is None:
        _NC_CACHE = _build()
    return _NC_CACHE


def _prep_in_maps(state, h, h_null, t, guidance_scale, W1, b1, W2, b2):
    f32 = np.float32
    xTf = np.ascontiguousarray(state[:, :DIM_X].T, dtype=f32)      # (128, B)
    hTf = np.ascontiguousarray(h.T, dtype=f32)
    hnTf = np.ascontiguousarray(h_null.T, dtype=f32)
    w1x = np.ascontiguousarray(W1[:DIM_X], dtype=f32)              # (128, 512)
    w1h = np.ascontiguousarray(W1[DIM_X:DIM_X + DIM_H], dtype=f32)
    b1p = (b1.astype(f32) + t.astype(f32)[0] * W1[DIM_X + DIM_H].astype(f32))
    w2r = np.ascontiguousarray(
        W2.astype(f32).reshape(NCH, 128, DIM_X).transpose(1, 0, 2).reshape(128, NCH * DIM_X))
    cvec = (w1x.astype(np.float64) * W2.astype(np.float64).T).sum(0)  # (512,)
    d0 = cvec.sum()
    cmatf = np.ascontiguousarray(cvec.reshape(NCH, 128).T, dtype=f32)  # (128, 4)
    gs = float(guidance_scale.astype(f32)[0])

    auxf = np.zeros((128, 10), f32)
    auxf[:, 0:4] = b1p.reshape(NCH, 128).T
    auxf[:, 4] = b2.astype(f32)
    auxf[:, 5] = gs
    auxf[:, 6] = 1.0 - gs
    auxf[:, 7] = -(1.0 - gs)
    auxf[:, 8] = d0
    auxf[:, 9] = -gs

    in_maps = []
    for i in range(N_CORES):
        sl = slice(i * R, (i + 1) * R)
        in_maps.append({
            "xT": np.ascontiguousarray(xTf[:, sl]),
            "hT": np.ascontiguousarray(hTf[:, sl]),
            "hnT": np.ascontiguousarray(hnTf[:, sl]),
            "w1x": w1x, "w1h": w1h, "w2t": w2r, "cmat": cmatf, "aux": auxf,
        })
    return in_maps


def kernel(state, h, h_null, t, guidance_scale, W1, b1, W2, b2, _trace=False):
    nc = _get_nc()
    in_maps = _prep_in_maps(state, h, h_null, t, guidance_scale, W1, b1, W2, b2)
    res = run_bass_kernel_spmd(nc, in_maps, list(range(N_CORES)), trace=_trace)
    out = np.empty((B, DIM_X + 1), np.float32)
    for i in range(N_CORES):
        sl = slice(i * R, (i + 1) * R)
        out[sl, :DIM_X] = res.results[i]["VO"].T
        out[sl, DIM_X] = res.results[i]["DO"][0]
    if _trace:
        return out, res
    return out
